# revision 61
# baseline (speedup 1.0000x reference)
"""CAM-style self-attention kernel for Trainium2 (8 NeuronCores, SPMD).

Reference computation (per batch sample b):
    q = x[b].reshape(N, C)                 # N = H*W = 4096, C = 512
    E = q @ q.T                            # [N, N]
    A = softmax(rowmax(E) - E, axis=-1)    # == exp(rowmin(E) - E) / rowsum
    out = A @ q
    y[b] = alpha * out + x[b]

Sharding: data-parallel over batch B=8 -> one sample per NeuronCore.

Implementation notes:
- Matmul operands are fp16 (1 PE cycle/row vs 4 for fp32); accumulation is
  fp32 in PSUM. Softmax here is extremely peaked, so fp16 E/P error is far
  below tolerance (verified at alpha=0.7 in test.py).
- All transposes (q -> qT for the E matmuls, P -> PT for the O matmuls) run
  on the DMA xbar (dma_start_transpose), not the PE: one [128,2048]
  transpose per 4-band quad builds qT16 in the xbar-native layout, and one
  per [128,2048] P chunk feeds the O matmuls. The PE stream is pure matmul:
  E (32x8x4) + O (32x32) at 512 free-elems each -> ~437us of work at 2.4
  GHz; this kernel reaches ~446us busy / 474us total.
- Queue discipline matters more than engine totals: x loads issue from the
  ACT/HWDGE queue (SWDGE's ~1.1us/issue Pool cost and cross-queue scheduler
  waits paced the load), y stores from ACT (on SP they head-of-line block
  the next band's P-transpose issues), xi prefetches from SWDGE.
- E rows are staged in SBUF as fp16 (halves DVE copy/reduce cost vs fp32).
  PSUM->SBUF copies alternate DVE/ACT; row-min partials interleave on DVE.
- Deep software pipeline: during the (DMA-bound) x-load window the PE
  eagerly computes E chunks for the first NEAGER rows; the main loop runs
  with an NEAGER-band lag (E(i) || softmax/O(i-NEAGER)), shrinking the lag
  near the end so the O-only tail stays short. P transposes are issued
  ahead of their O matmuls to hide the ~3us DMA-transpose latency.
"""

import numpy as np

import concourse.bass as bass
import concourse.mybir as mybir
import concourse.tile as tile
from concourse.bass_utils import run_bass_kernel_spmd

B, H, W, C = 8, 64, 64, 512
N = H * W            # 4096
P = 128              # partitions
NT = N // P          # 32 row bands
KC = C // P          # 4 contraction chunks for E (K = C = 512)
CH = 512             # free-dim chunk (one PSUM bank of fp32)
NCH = N // CH        # 8 chunks per row band
PCH = 2048           # P-phase exp/transpose chunk (4 PSUM-chunk widths)
NPCH = N // PCH      # 2 P-chunks per band

F32 = mybir.dt.float32
F16 = mybir.dt.float16

_CACHE = {}
LAST_RESULTS = None  # stashed BassKernelResults for test harness introspection


def _build_bass():
    nc = bass.Bass()
    x_d = nc.declare_dram_parameter("x", [N, C], F32, isOutput=False)
    a_d = nc.declare_dram_parameter("alpha", [1, 1], F32, isOutput=False)
    y_d = nc.declare_dram_parameter("y", [N, C], F32, isOutput=True)

    with tile.TileContext(nc) as tc:
        with (
            tc.tile_pool(name="persist", bufs=1) as persist,
            tc.tile_pool(name="ework", bufs=2) as ework,
            tc.tile_pool(name="small", bufs=4) as small,
            tc.tile_pool(name="outp", bufs=2) as outp,
            tc.tile_pool(name="stats", bufs=4) as stats,
            tc.tile_pool(name="psum", bufs=2, space="PSUM") as psum,
        ):
            # ---- persistent tiles ----
            q16 = persist.tile([P, NT, C], F16)     # fp16 copy (matmul rhs)
            # transposed q, xbar-native layout: one [128,2048] DMA transpose
            # per 4-band quad. qT16[p, g, a, f] = q[(4g + a//4)*128 + f,
            # (a%4)*128 + p]
            qT16 = persist.tile([P, NT // 4, 16, P], F16)
            alpha_sb = persist.tile([P, 1], F32)

            saved = {}
            NEAGER = 8  # rows with eager E chunks during the load phase

            rmin4s = {}

            qT_t = qT16.tensor

            def qT_lhs(i, k):
                # band i, chunk k
                return qT16[:, i // 4, (i % 4) * 4 + k, :]

            def qT_rhs(j, k):
                # cols quad j, chunk k: a in {k, 4+k, 8+k, 12+k} (stride
                # 4*128 elements), 128 wide each
                return bass.AP(
                    tensor=qT_t, offset=qT16.offset + (j * 16 + k) * P,
                    ap=[list(qT16.ap[0]), [4 * P, 4], [1, P]])

            def e_chunk(e_sb, i, j):
                ep = psum.tile([P, CH], F32, tag="e", bufs=6)
                for k in range(KC):
                    nc.tensor.matmul(
                        ep,
                        qT_lhs(i, k),
                        qT_rhs(j, k),
                        start=(k == 0),
                        stop=(k == KC - 1),
                    )
                # PSUM f32 -> SBUF fp16 row stage, alternating DVE/ACT
                # (GPSIMD cannot read PSUM); row-min partials are split so
                # the DVE never stalls the bank drain for long
                if j % 2 == 0:
                    nc.vector.tensor_copy(e_sb[:, j * CH:(j + 1) * CH], ep)
                else:
                    nc.scalar.copy(e_sb[:, j * CH:(j + 1) * CH], ep)
                if j % 2 == 1:
                    # row-min partial over the completed [128,1024] pair
                    if (i, 0) not in rmin4s:
                        rmin4s[(i, 0)] = stats.tile(
                            [P, NCH // 2], F32, name=f"rm4_{i}", tag="rmin4",
                            bufs=NEAGER + 1)
                    nc.vector.tensor_reduce(
                        rmin4s[(i, 0)][:, j // 2:j // 2 + 1],
                        e_sb[:, (j - 1) * CH:(j + 1) * CH],
                        axis=mybir.AxisListType.X, op=mybir.AluOpType.min)

            def e_finish(e_sb, i):
                rmin = stats.tile([P, 1], F32, tag="rmin", bufs=NEAGER + 1)
                nc.vector.tensor_reduce(
                    rmin, rmin4s.pop((i, 0)), axis=mybir.AxisListType.X,
                    op=mybir.AluOpType.min)
                saved[i] = (e_sb, rmin)

            # ---- load x (= q), round to fp16, build qT via DMA transposes.
            # While the (DMA-bound) load streams in, eagerly compute E
            # chunks for the first NEAGER rows as their qT column groups
            # land, so the PE is never starved during the load window. ----
            rows = {}

            def get_row(i):
                if i not in rows:
                    rows[i] = ework.tile([P, N], F16, name=f"e{i}",
                                         tag="e", bufs=NEAGER)
                return rows[i]

            for g in range(8):  # 8 DMA groups x 4 row bands
                # Tile enforces a global in-flight window of ~8 DMAs (DMA #n
                # waits on #n-8), so the first 4 qT transposes must land
                # within the first window: band-by-band x loads for g=0,
                # each followed immediately by its transpose; alpha and the
                # remaining quad loads come after.
                bsl = 2 if g == 0 else 4
                import contextlib
                prio = (tc.high_priority() if g == 0
                        else contextlib.nullcontext())
                with prio:
                    for b0 in range(4 * g, 4 * (g + 1), bsl):
                        sl = slice(b0, b0 + bsl)
                        stage = small.tile([P, 4, C], F32, tag="xs", bufs=4)
                        # issue x loads from the ACT/HWDGE queue ahead
                        # of the convs (SP would head-of-line block behind
                        # transpose waits)
                        nc.scalar.dma_start(
                            out=stage[:, :bsl, :],
                            in_=x_d[b0 * P:(b0 + bsl) * P, :].rearrange(
                                "(i p) c -> p i c", p=P),
                        )
                        nc.scalar.copy(q16[:, sl, :], stage[:, :bsl, :])
                    # one whole-quad xbar transpose:
                    # in [128, 2048] -> out [128, 16, 128]
                    nc.sync.dma_start_transpose(
                        out=qT16[:, g, :, :],
                        in_=q16[:, 4 * g:4 * (g + 1), :],
                    )
                if g == 0:
                    # broadcast-load alpha across all partitions (needed
                    # only by the first p_phase, far later)
                    a_ap = a_d[:, :]
                    a_bc = bass.AP(tensor=a_ap.tensor, offset=a_ap.offset,
                                   ap=[[0, P], [1, 1]])
                    nc.gpsimd.dma_start(out=alpha_sb, in_=a_bc)
                # qT col-group g is now in flight. Eager rows emit every
                # chunk (i, j) that just became computable: column g for
                # rows already started, columns 0..g for rows that just
                # became available.
                prev_hi = min(4 * g, NEAGER) if g > 0 else 0
                hi = min(4 * g + 4, NEAGER)
                for i in range(hi):
                    e_sb = get_row(i)
                    for j in (range(g, g + 1) if i < prev_hi
                              else range(0, g + 1)):
                        e_chunk(e_sb, i, j)
                    if g == 7:
                        e_finish(e_sb, i)

            # ---- main loop, software-pipelined with NEAGER-band lag ----
            def e_phase(i):
                e_sb = get_row(i)
                for j in range(NCH):
                    e_chunk(e_sb, i, j)
                e_finish(e_sb, i)

            def p_phase(i):
                e_sb, rmin = saved.pop(i)
                # prefetch x band i for the residual add (q32 is not kept
                # resident; DMA has ample headroom)
                xi = outp.tile([P, C], F32, tag="xi", bufs=3)
                nc.gpsimd.dma_start(out=xi, in_=x_d[i * P:(i + 1) * P, :])
                zparts = stats.tile([P, NPCH], F32, tag="z")
                o_ps = psum.tile([P, C], F32, tag="o", bufs=2)

                def exp_T(j):
                    p16 = small.tile([P, PCH], F16, tag="p", bufs=3)
                    nc.scalar.activation(
                        p16, e_sb[:, j * PCH:(j + 1) * PCH],
                        mybir.ActivationFunctionType.Exp,
                        bias=rmin, scale=-1.0,
                        accum_out=zparts[:, j:j + 1],
                    )
                    pt16 = small.tile([P, PCH // P, P], F16, tag="pt", bufs=4)
                    # pt16[p, jj, f] = p16[f, jj*128+p]  (lhsT chunks for O)
                    nc.sync.dma_start_transpose(out=pt16, in_=p16)
                    return pt16

                def o_mm(j, pt16):
                    for jj in range(PCH // P):
                        m = (PCH // P) * j + jj
                        nc.tensor.matmul(
                            o_ps,
                            pt16[:, jj, :],
                            q16[:, m, :],
                            start=(m == 0),
                            stop=(m == NT - 1),
                        )

                # transposes run two chunks ahead of the O matmuls so the DMA
                # transpose latency is hidden behind PE work
                pts = [exp_T(0)] + ([exp_T(1)] if NPCH > 1 else [])
                for j in range(NPCH):
                    if j + 2 < NPCH:
                        pts.append(exp_T(j + 2))
                    o_mm(j, pts[j])
                z = stats.tile([P, 1], F32, tag="zs")
                nc.vector.reduce_sum(z, zparts, axis=mybir.AxisListType.X)
                rz = stats.tile([P, 1], F32, tag="rz")
                nc.vector.reciprocal(rz, z)
                s = stats.tile([P, 1], F32, tag="s")
                nc.vector.tensor_mul(s, rz, alpha_sb)
                o_sb = outp.tile([P, C], F32, tag="o")
                nc.scalar.mul(o_sb, o_ps, mul=s)
                yt = outp.tile([P, C], F32, tag="y")
                nc.vector.tensor_add(yt, o_sb, xi)
                # issue the store from the ACT queue: on SP it would block
                # the next band's P-transpose issues behind the yt wait
                nc.scalar.dma_start(out=y_d[i * P:(i + 1) * P, :], in_=yt)

            # shrink the pipeline lag from NEAGER to 2 near the end so the
            # O-only tail (no E work to hide producer latency) is short
            p_next = 0
            for i in range(NEAGER, NT):
                e_phase(i)
                p_phase(p_next)
                p_next += 1
                if i >= NT - 8:
                    p_phase(p_next)
                    p_next += 1
            while p_next < NT:
                p_phase(p_next)
                p_next += 1

    _split_matmul_waits(nc)
    return nc


def _split_matmul_waits(nc):
    """Several TRN2 instruction structs (Matmult/Ldweights self-loading path,
    Activation) carry at most ONE sync wait; Tile sometimes emits more. Fix
    by inserting same-engine NoOps immediately before the offender, each
    carrying one surplus wait. A wait moved onto the directly-preceding
    instruction of the same engine is strictly more conservative, so safe."""
    import bass_rust

    LIMITED = {"InstMatmult", "InstLdweights", "InstActivation",
               "InstDmaTransposeAnt", "InstTensorTensor", "InstTensorCopy",
               "InstTensorReduce", "InstReciprocal", "InstTensorScalarPtr",
               "InstTensorScalarAffineSelect", "InstMemset", "InstIota",
               "InstCopyPredicated", "InstTensorScalar", "InstDMACopy",
               "InstDrain"}
    n_nops = 0
    for bb in nc.m.functions[0].blocks:
        insts = list(bb.instructions)
        out = []
        for inst in insts:
            tn = type(inst).__name__
            si = inst.sync_info
            waits = list(si.on_wait) if si else []
            if tn in LIMITED and len(waits) > 1:
                # if directly preceded by this matmul's Ldweights, put the
                # nops before the LDW to keep the LDW+MM pair adjacent
                ins_at = len(out)
                if (tn == "InstMatmult" and out
                        and type(out[-1]).__name__ == "InstLdweights"):
                    ins_at = len(out) - 1
                for w in waits[:-1]:
                    nop = bass_rust.InstNoOp(
                        name=f"I-waitfix-{n_nops}", ins=[], outs=[])
                    nop.engine = inst.engine
                    nop.sync_info = mybir.SyncInfo(on_wait=[w], on_update=[])
                    out.insert(ins_at, nop)
                    ins_at += 1
                    n_nops += 1
                inst.sync_info = mybir.SyncInfo(
                    on_wait=waits[-1:], on_update=list(si.on_update))
            out.append(inst)
        if len(out) != len(insts):
            bb.instructions = out
    return n_nops


def kernel(x, alpha):
    global LAST_RESULTS
    import os
    import time
    # This environment has no NTFF profiling hook (antenv.axon_hooks); a set
    # BASS_TRACE would crash the axon redirect, so force the no-trace path.
    os.environ.setdefault("BASS_NEVER_TRACE", "1")

    x = np.asarray(x, dtype=np.float32)
    alpha = np.asarray(alpha, dtype=np.float32)
    if "nc" not in _CACHE:
        _CACHE["nc"] = _build_bass()
    nc = _CACHE["nc"]

    in_maps = [
        {"x": np.ascontiguousarray(x[b].reshape(N, C)),
         "alpha": alpha.reshape(1, 1)}
        for b in range(B)
    ]
    res = None
    for attempt in range(3):
        try:
            res = run_bass_kernel_spmd(nc, in_maps, list(range(B)))
            break
        except Exception:
            # transient NRT/axon device errors have been observed; retry
            if attempt == 2:
                raise
            time.sleep(5)
    LAST_RESULTS = res
    out = np.stack([res.results[b]["y"].reshape(H, W, C) for b in range(B)])
    return out


# revision 68
# speedup vs baseline: 1.0065x; 1.0065x over previous
"""CAM-style self-attention kernel for Trainium2 (8 NeuronCores, SPMD).

Reference computation (per batch sample b):
    q = x[b].reshape(N, C)                 # N = H*W = 4096, C = 512
    E = q @ q.T                            # [N, N]
    A = softmax(rowmax(E) - E, axis=-1)    # == exp(rowmin(E) - E) / rowsum
    out = A @ q
    y[b] = alpha * out + x[b]

Sharding: data-parallel over batch B=8 -> one sample per NeuronCore.

Implementation notes:
- Matmul operands are fp16 (1 PE cycle/row vs 4 for fp32); accumulation is
  fp32 in PSUM. Softmax here is extremely peaked, so fp16 E/P error is far
  below tolerance (verified at alpha=0.7 in test.py).
- All transposes (q -> qT for the E matmuls, P -> PT for the O matmuls) run
  on the DMA xbar (dma_start_transpose), not the PE: one [128,2048]
  transpose per 4-band quad builds qT16 in the xbar-native layout, and one
  per [128,2048] P chunk feeds the O matmuls. The PE stream is pure matmul:
  E (32x8x4) + O (32x32) at 512 free-elems each -> ~437us of work at 2.4
  GHz; this kernel reaches ~446us busy / 474us total.
- Queue discipline matters more than engine totals: x loads issue from the
  ACT/HWDGE queue (SWDGE's ~1.1us/issue Pool cost and cross-queue scheduler
  waits paced the load), y stores from ACT (on SP they head-of-line block
  the next band's P-transpose issues), xi prefetches from SWDGE.
- E rows are staged in SBUF as fp16 (halves DVE copy/reduce cost vs fp32).
  PSUM->SBUF copies alternate DVE/ACT; row-min partials interleave on DVE.
- Deep software pipeline: during the (DMA-bound) x-load window the PE
  eagerly computes E chunks for the first NEAGER rows; the main loop runs
  with an NEAGER-band lag (E(i) || softmax/O(i-NEAGER)), shrinking the lag
  near the end so the O-only tail stays short. P transposes are issued
  ahead of their O matmuls to hide the ~3us DMA-transpose latency.
"""

import numpy as np

import concourse.bass as bass
import concourse.mybir as mybir
import concourse.tile as tile
from concourse.bass_utils import run_bass_kernel_spmd

B, H, W, C = 8, 64, 64, 512
N = H * W            # 4096
P = 128              # partitions
NT = N // P          # 32 row bands
KC = C // P          # 4 contraction chunks for E (K = C = 512)
CH = 512             # free-dim chunk (one PSUM bank of fp32)
NCH = N // CH        # 8 chunks per row band
PCH = 2048           # P-phase exp/transpose chunk (4 PSUM-chunk widths)
NPCH = N // PCH      # 2 P-chunks per band

F32 = mybir.dt.float32
F16 = mybir.dt.float16

_CACHE = {}
LAST_RESULTS = None  # stashed BassKernelResults for test harness introspection


def _build_bass():
    nc = bass.Bass()
    x_d = nc.declare_dram_parameter("x", [N, C], F32, isOutput=False)
    a_d = nc.declare_dram_parameter("alpha", [1, 1], F32, isOutput=False)
    y_d = nc.declare_dram_parameter("y", [N, C], F32, isOutput=True)

    with tile.TileContext(nc) as tc:
        with (
            tc.tile_pool(name="persist", bufs=1) as persist,
            tc.tile_pool(name="ework", bufs=2) as ework,
            tc.tile_pool(name="small", bufs=4) as small,
            tc.tile_pool(name="outp", bufs=2) as outp,
            tc.tile_pool(name="stats", bufs=4) as stats,
            tc.tile_pool(name="psum", bufs=2, space="PSUM") as psum,
        ):
            # ---- persistent tiles ----
            q16 = persist.tile([P, NT, C], F16)     # fp16 copy (matmul rhs)
            # transposed q, xbar-native layout: one [128,2048] DMA transpose
            # per 4-band quad. qT16[p, g, a, f] = q[(4g + a//4)*128 + f,
            # (a%4)*128 + p]
            qT16 = persist.tile([P, NT // 4, 16, P], F16)
            alpha_sb = persist.tile([P, 1], F32)

            saved = {}
            NEAGER = 8  # rows with eager E chunks during the load phase

            rmin4s = {}

            qT_t = qT16.tensor

            def qT_lhs(i, k):
                # band i, chunk k
                return qT16[:, i // 4, (i % 4) * 4 + k, :]

            def qT_rhs(j, k):
                # cols quad j, chunk k: a in {k, 4+k, 8+k, 12+k} (stride
                # 4*128 elements), 128 wide each
                return bass.AP(
                    tensor=qT_t, offset=qT16.offset + (j * 16 + k) * P,
                    ap=[list(qT16.ap[0]), [4 * P, 4], [1, P]])

            def _rm4(i):
                if (i, 0) not in rmin4s:
                    rmin4s[(i, 0)] = stats.tile(
                        [P, NCH // 2], F32, name=f"rm4_{i}", tag="rmin4",
                        bufs=NEAGER + 1)
                return rmin4s[(i, 0)]

            def _partial(e_sb, i, j):
                # row-min partial over the completed [128,1024] pair j-1,j
                nc.vector.tensor_reduce(
                    _rm4(i)[:, j // 2:j // 2 + 1],
                    e_sb[:, (j - 1) * CH:(j + 1) * CH],
                    axis=mybir.AxisListType.X, op=mybir.AluOpType.min)

            def e_chunk(e_sb, i, j):
                ep = psum.tile([P, CH], F32, tag="e", bufs=6)
                for k in range(KC):
                    nc.tensor.matmul(
                        ep,
                        qT_lhs(i, k),
                        qT_rhs(j, k),
                        start=(k == 0),
                        stop=(k == KC - 1),
                    )
                # PSUM f32 -> SBUF fp16 row stage. During the load window
                # the eager rows' copies all ride DVE so the ACT queue
                # stays a pure x-load/convert stream (ACT copies wait on PE
                # matmuls and would head-of-line block the convs, pacing
                # the qT delivery the PE itself is waiting on). Main-loop
                # bands alternate DVE/ACT; partials interleave on DVE.
                if i < NEAGER:
                    # load window: mostly-DVE copies (1 in 4 on ACT) so the
                    # ACT x/conv stream stays nearly unblocked
                    if (i + j) % 4 == 3:
                        nc.scalar.copy(e_sb[:, j * CH:(j + 1) * CH], ep)
                    else:
                        nc.vector.tensor_copy(
                            e_sb[:, j * CH:(j + 1) * CH], ep)
                else:
                    if j % 2 == 0:
                        nc.vector.tensor_copy(
                            e_sb[:, j * CH:(j + 1) * CH], ep)
                    else:
                        nc.scalar.copy(e_sb[:, j * CH:(j + 1) * CH], ep)
                    if j % 2 == 1:
                        _partial(e_sb, i, j)

            def e_finish(e_sb, i):
                if i < NEAGER:
                    # eager rows: partials postponed past the load window
                    # (DVE is copy-saturated there); lag hides the latency
                    for j in (1, 3, 5, 7):
                        _partial(e_sb, i, j)
                rmin = stats.tile([P, 1], F32, tag="rmin", bufs=NEAGER + 1)
                nc.vector.tensor_reduce(
                    rmin, rmin4s.pop((i, 0)), axis=mybir.AxisListType.X,
                    op=mybir.AluOpType.min)
                saved[i] = (e_sb, rmin)

            # ---- load x (= q), round to fp16, build qT via DMA transposes.
            # While the (DMA-bound) load streams in, eagerly compute E
            # chunks for the first NEAGER rows as their qT column groups
            # land, so the PE is never starved during the load window. ----
            rows = {}

            def get_row(i):
                if i not in rows:
                    rows[i] = ework.tile([P, N], F16, name=f"e{i}",
                                         tag="e", bufs=NEAGER)
                return rows[i]

            for g in range(8):  # 8 DMA groups x 4 row bands
                # Tile enforces a global in-flight window of ~8 DMAs (DMA #n
                # waits on #n-8), so the first 4 qT transposes must land
                # within the first window: band-by-band x loads for g=0,
                # each followed immediately by its transpose; alpha and the
                # remaining quad loads come after.
                bsl = 2 if g == 0 else 4
                import contextlib
                prio = (tc.high_priority() if g == 0
                        else contextlib.nullcontext())
                with prio:
                    for b0 in range(4 * g, 4 * (g + 1), bsl):
                        sl = slice(b0, b0 + bsl)
                        stage = small.tile([P, 4, C], F32, tag="xs", bufs=4)
                        # issue x loads from the ACT/HWDGE queue ahead
                        # of the convs (SP would head-of-line block behind
                        # transpose waits)
                        nc.scalar.dma_start(
                            out=stage[:, :bsl, :],
                            in_=x_d[b0 * P:(b0 + bsl) * P, :].rearrange(
                                "(i p) c -> p i c", p=P),
                        )
                        nc.scalar.copy(q16[:, sl, :], stage[:, :bsl, :])
                    # one whole-quad xbar transpose:
                    # in [128, 2048] -> out [128, 16, 128]
                    nc.sync.dma_start_transpose(
                        out=qT16[:, g, :, :],
                        in_=q16[:, 4 * g:4 * (g + 1), :],
                    )
                if g == 0:
                    # broadcast-load alpha across all partitions (needed
                    # only by the first p_phase, far later)
                    a_ap = a_d[:, :]
                    a_bc = bass.AP(tensor=a_ap.tensor, offset=a_ap.offset,
                                   ap=[[0, P], [1, 1]])
                    nc.gpsimd.dma_start(out=alpha_sb, in_=a_bc)
                # qT col-group g is now in flight. Eager rows emit every
                # chunk (i, j) that just became computable: column g for
                # rows already started, columns 0..g for rows that just
                # became available.
                prev_hi = min(4 * g, NEAGER) if g > 0 else 0
                hi = min(4 * g + 4, NEAGER)
                for i in range(hi):
                    e_sb = get_row(i)
                    for j in (range(g, g + 1) if i < prev_hi
                              else range(0, g + 1)):
                        e_chunk(e_sb, i, j)
                    if g == 7:
                        e_finish(e_sb, i)

            # ---- main loop, software-pipelined with NEAGER-band lag ----
            def e_phase(i):
                e_sb = get_row(i)
                for j in range(NCH):
                    e_chunk(e_sb, i, j)
                e_finish(e_sb, i)

            def p_phase(i):
                e_sb, rmin = saved.pop(i)
                # prefetch x band i for the residual add (q32 is not kept
                # resident; DMA has ample headroom)
                xi = outp.tile([P, C], F32, tag="xi", bufs=3)
                nc.gpsimd.dma_start(out=xi, in_=x_d[i * P:(i + 1) * P, :])
                zparts = stats.tile([P, NPCH], F32, tag="z")
                o_ps = psum.tile([P, C], F32, tag="o", bufs=2)

                def exp_T(j):
                    p16 = small.tile([P, PCH], F16, tag="p", bufs=3)
                    nc.scalar.activation(
                        p16, e_sb[:, j * PCH:(j + 1) * PCH],
                        mybir.ActivationFunctionType.Exp,
                        bias=rmin, scale=-1.0,
                        accum_out=zparts[:, j:j + 1],
                    )
                    pt16 = small.tile([P, PCH // P, P], F16, tag="pt", bufs=4)
                    # pt16[p, jj, f] = p16[f, jj*128+p]  (lhsT chunks for O)
                    nc.sync.dma_start_transpose(out=pt16, in_=p16)
                    return pt16

                def o_mm(j, pt16):
                    for jj in range(PCH // P):
                        m = (PCH // P) * j + jj
                        nc.tensor.matmul(
                            o_ps,
                            pt16[:, jj, :],
                            q16[:, m, :],
                            start=(m == 0),
                            stop=(m == NT - 1),
                        )

                # transposes run two chunks ahead of the O matmuls so the DMA
                # transpose latency is hidden behind PE work
                pts = [exp_T(0)] + ([exp_T(1)] if NPCH > 1 else [])
                for j in range(NPCH):
                    if j + 2 < NPCH:
                        pts.append(exp_T(j + 2))
                    o_mm(j, pts[j])
                z = stats.tile([P, 1], F32, tag="zs")
                nc.vector.reduce_sum(z, zparts, axis=mybir.AxisListType.X)
                rz = stats.tile([P, 1], F32, tag="rz")
                nc.vector.reciprocal(rz, z)
                s = stats.tile([P, 1], F32, tag="s")
                nc.vector.tensor_mul(s, rz, alpha_sb)
                o_sb = outp.tile([P, C], F32, tag="o")
                nc.scalar.mul(o_sb, o_ps, mul=s)
                yt = outp.tile([P, C], F32, tag="y")
                nc.vector.tensor_add(yt, o_sb, xi)
                # issue the store from the ACT queue: on SP it would block
                # the next band's P-transpose issues behind the yt wait
                nc.scalar.dma_start(out=y_d[i * P:(i + 1) * P, :], in_=yt)

            # shrink the pipeline lag from NEAGER to 2 near the end so the
            # O-only tail (no E work to hide producer latency) is short
            p_next = 0
            for i in range(NEAGER, NT):
                e_phase(i)
                p_phase(p_next)
                p_next += 1
                if i >= NT - 8:
                    p_phase(p_next)
                    p_next += 1
            while p_next < NT:
                p_phase(p_next)
                p_next += 1

    _split_matmul_waits(nc)
    return nc


def _split_matmul_waits(nc):
    """Several TRN2 instruction structs (Matmult/Ldweights self-loading path,
    Activation) carry at most ONE sync wait; Tile sometimes emits more. Fix
    by inserting same-engine NoOps immediately before the offender, each
    carrying one surplus wait. A wait moved onto the directly-preceding
    instruction of the same engine is strictly more conservative, so safe."""
    import bass_rust

    LIMITED = {"InstMatmult", "InstLdweights", "InstActivation",
               "InstDmaTransposeAnt", "InstTensorTensor", "InstTensorCopy",
               "InstTensorReduce", "InstReciprocal", "InstTensorScalarPtr",
               "InstTensorScalarAffineSelect", "InstMemset", "InstIota",
               "InstCopyPredicated", "InstTensorScalar", "InstDMACopy",
               "InstDrain"}
    n_nops = 0
    for bb in nc.m.functions[0].blocks:
        insts = list(bb.instructions)
        out = []
        for inst in insts:
            tn = type(inst).__name__
            si = inst.sync_info
            waits = list(si.on_wait) if si else []
            if tn in LIMITED and len(waits) > 1:
                # if directly preceded by this matmul's Ldweights, put the
                # nops before the LDW to keep the LDW+MM pair adjacent
                ins_at = len(out)
                if (tn == "InstMatmult" and out
                        and type(out[-1]).__name__ == "InstLdweights"):
                    ins_at = len(out) - 1
                for w in waits[:-1]:
                    nop = bass_rust.InstNoOp(
                        name=f"I-waitfix-{n_nops}", ins=[], outs=[])
                    nop.engine = inst.engine
                    nop.sync_info = mybir.SyncInfo(on_wait=[w], on_update=[])
                    out.insert(ins_at, nop)
                    ins_at += 1
                    n_nops += 1
                inst.sync_info = mybir.SyncInfo(
                    on_wait=waits[-1:], on_update=list(si.on_update))
            out.append(inst)
        if len(out) != len(insts):
            bb.instructions = out
    return n_nops


def kernel(x, alpha):
    global LAST_RESULTS
    import os
    import time
    # This environment has no NTFF profiling hook (antenv.axon_hooks); a set
    # BASS_TRACE would crash the axon redirect, so force the no-trace path.
    os.environ.setdefault("BASS_NEVER_TRACE", "1")

    x = np.asarray(x, dtype=np.float32)
    alpha = np.asarray(alpha, dtype=np.float32)
    if "nc" not in _CACHE:
        _CACHE["nc"] = _build_bass()
    nc = _CACHE["nc"]

    in_maps = [
        {"x": np.ascontiguousarray(x[b].reshape(N, C)),
         "alpha": alpha.reshape(1, 1)}
        for b in range(B)
    ]
    res = None
    for attempt in range(3):
        try:
            res = run_bass_kernel_spmd(nc, in_maps, list(range(B)))
            break
        except Exception:
            # transient NRT/axon device errors have been observed; retry
            if attempt == 2:
                raise
            time.sleep(5)
    LAST_RESULTS = res
    out = np.stack([res.results[b]["y"].reshape(H, W, C) for b in range(B)])
    return out
